# revision 113
# baseline (speedup 1.0000x reference)
"""Trainium2 Bass kernel for a small autoregressive transformer block with
local-windowed causal attention and a large (16k) vocab head.

Data-parallel over batch: batch item b runs on NeuronCore b (8 cores).
Per core (host precomputes h = emb[x]+pos, A = WqWk^T/sqrt(D), B = WvWo):
  s    = (h A) h^T + mask      banded (window <= 298), bf16 matmuls
  o    = softmax(s) @ (h B)    bf16; softmax-normalize fused into residual
  h1   = LN(h + o);  f = relu(h1@W1)@W2;  h2 = LN(h1 + f)   (bf16 matmuls)
  out  = h2 @ Wh               fp8-e4m3 DoubleRow with first-order error
                               correction: xh@wh + xl@wh + xh@wl, where
                               x = SX*h2 (folded into LN2), w = SW*Wh (host)

The vocab head is ~70%% of PE time; error-corrected fp8 DoubleRow runs it at
1.33x the bf16 rate with bf16-level accuracy (rel err ~4e-3 vs 2e-2 gate).
kernel(**inputs) takes full unsharded inputs, returns [8, 1024, 16384] f32.
"""

import math
import numpy as np

import concourse.bass as bass
import concourse.mybir as mybir
import concourse.tile as tile
from concourse import bacc
from concourse.bass_utils import run_bass_kernel_spmd
from concourse.masks import make_identity

# ---- problem constants (hardcoded per contract) ----
GH = 32
GW = 32
SEQ = 1024
WIN = 9
D = 512
DFF = 1024
VOCAB = 16384
EPS = 1e-5
NEG = -240.0  # exact in fp8-e4m3; exp(s + NEG) underflows to 0

P = 128
NT = SEQ // P        # 8 token chunks
DC = D // P          # 4 d chunks
FC = DFF // P        # 8 dff chunks
NV = VOCAB // 512    # 32 vocab chunks

F32 = mybir.dt.float32
BF16 = mybir.dt.bfloat16
F8 = mybir.dt.float8e4
OUT_BF16 = True
AF = mybir.ActivationFunctionType
DR = mybir.MatmulPerfMode.DoubleRow

# error-corrected fp8 head: logits = xh@wh + xl@wh + xh@wl, DoubleRow matmuls.
# h2 is produced pre-scaled by SX (folded into LN2's rsqrt); Wh is pre-scaled
# by SW on the host; the eviction copy divides by SX*SW.
SX = 8.0
SW = 32.0
INV_SXSW = 1.0 / (SX * SW)


def _window_start(i: int) -> int:
    # k-window [ws, ws+512) covers all allowed keys for query chunk i
    # (max lookback is WIN*GW + WIN = 297 < 384).
    return 128 * max(0, i - 3)


def _mask_tiles() -> np.ndarray:
    idx = np.arange(SEQ)
    r, c = idx // GW, idx % GW
    allow = (
        (np.abs(r[:, None] - r[None, :]) <= WIN)
        & (np.abs(c[:, None] - c[None, :]) <= WIN)
        & (idx[None, :] <= idx[:, None])
    )
    maskf = np.where(allow, 0.0, NEG).astype(np.float32)
    tiles = np.empty((NT, P, 512), np.float32)
    for i in range(NT):
        ws = _window_start(i)
        tiles[i] = maskf[i * P : (i + 1) * P, ws : ws + 512]
    return tiles


def _bcast_ap(a: bass.AP) -> bass.AP:
    """[n] DRAM vector AP -> [P, n] partition-broadcast DMA source."""
    return bass.AP(tensor=a.tensor, offset=a.offset, ap=[[0, P], *a.ap])


def _build_program(flags: dict, wh_bufs: int = 16) -> bass.Bass:
    nc = bacc.Bacc("TRN2", target_bir_lowering=False)

    # ---------- I/O ----------
    # h = emb[x] + pos is gathered host-side and shipped in both layouts:
    # token-major (residual adds, f32) and d-major/transposed (matmul
    # operand, bf16 — quantization is well under the error budget)
    h_d = nc.declare_dram_parameter("h", [P, NT, D], F32, False)
    ht_d = nc.declare_dram_parameter("ht", [P, DC, SEQ], BF16, False)
    msk_d = nc.declare_dram_parameter("maskt", [NT, P, 512], F8, False)
    # fused attention weights: A = Wq @ Wk^T / sqrt(D)  (scores = hA h^T),
    # B = Wv @ Wo  (o = attn @ (h B)); both computed host-side, shipped bf16.
    # wa is shipped m-major [P, m, ki, col] so per-column-block DMAs
    # have 1KB-contiguous descriptors
    wa_d = nc.declare_dram_parameter("wa", [P, DC, DC, P], BF16, False)
    wb_d = nc.declare_dram_parameter("wb", [D, D], BF16, False)
    w1_d = nc.declare_dram_parameter("w1", [D, DFF], BF16, False)
    w2_d = nc.declare_dram_parameter("w2", [DFF, D], BF16, False)
    whh_d = nc.declare_dram_parameter("whh", [D, VOCAB], F8, False)
    whl_d = nc.declare_dram_parameter("whl", [D, VOCAB], F8, False)
    dp = lambda name, shape: nc.declare_dram_parameter(name, shape, F32, False)
    assert not (flags["bq"] or flags["bk"]), (
        "QK-fused path requires zero q/k biases (true for this problem)"
    )
    # bc = bv @ Wo + bo, folded into the attention-output residual add
    bc_d = dp("bc", [D]) if flags["bc"] else None
    b1_d = dp("b1", [DFF]) if flags["b1"] else None
    b2_d = dp("b2", [D]) if flags["b2"] else None
    bh_d = dp("bh", [VOCAB]) if flags["bh"] else None
    g1_d = dp("g1", [D]) if flags["g1"] else None
    be1_d = dp("be1", [D]) if flags["be1"] else None
    g2_d = dp("g2", [D]) if flags["g2"] else None
    be2_d = dp("be2", [D]) if flags["be2"] else None
    out_d = nc.declare_dram_parameter("out", [SEQ, VOCAB], BF16 if OUT_BF16 else F32, True)

    with tile.TileContext(nc) as tc:
        # ----- whole-kernel pools -----
        const = tc.alloc_tile_pool(name="const", bufs=1)
        small = tc.alloc_tile_pool(name="small", bufs=8)
        psum = tc.alloc_tile_pool(name="psA", bufs=5, space="PSUM")
        psum_t = tc.alloc_tile_pool(name="psT", bufs=2, space="PSUM")
        opool = tc.alloc_tile_pool(name="outev", bufs=2, side="right")
        p_h2T = tc.alloc_tile_pool(name="h2Tp", bufs=1, side="right")

        ident_f = const.tile([P, P], F32, tag="ident_f")
        eps_t = const.tile([P, 1], F32, tag="eps")
        nc.vector.memset(eps_t[:], EPS)
        # eps for LN2 with the SX scale folded in: sqrt((var+eps)/SX^2)
        eps2_t = const.tile([P, 1], F32, tag="eps2")
        nc.vector.memset(eps2_t[:], EPS / (SX * SX))

        def load_col_bias(handle, nchunks, tag):
            # [nchunks*P] DRAM -> [P, nchunks] (chunk m in column m)
            t = const.tile([P, nchunks], F32, tag=tag)
            nc.sync.dma_start(out=t[:], in_=handle[:].rearrange("(m p) -> p m", p=P))
            return t

        def load_bcast(handle, n, tag):
            t = const.tile([P, n], F32, tag=tag)
            nc.sync.dma_start(out=t[:], in_=_bcast_ap(handle[:]))
            return t

        b1_sb = load_col_bias(b1_d, FC, "b1") if b1_d else None
        bc_bc = load_bcast(bc_d, D, "bc") if bc_d else None
        b2_bc = load_bcast(b2_d, D, "b2") if b2_d else None
        g1_bc = load_bcast(g1_d, D, "g1") if g1_d else None
        be1_bc = load_bcast(be1_d, D, "be1") if be1_d else None
        g2_bc = load_bcast(g2_d, D, "g2") if g2_d else None
        be2_bc = load_bcast(be2_d, D, "be2") if be2_d else None

        h2Th = [p_h2T.tile([P, DC, P], F8, tag=f"h2Th{j}", name=f"h2Th{j}") for j in range(NT)]
        h2Tl = [p_h2T.tile([P, DC, P], F8, tag=f"h2Tl{j}", name=f"h2Tl{j}") for j in range(NT)]

        # ----- phase A pools (left, LIFO) -----
        p_woh = tc.alloc_tile_pool(name="woh", bufs=1)         # h  (-> stage 4)
        h_sb = p_woh.tile([P, NT, D], F32, tag="h")

        p_v = tc.alloc_tile_pool(name="vp", bufs=1)            # v (-> wave 2)
        v_sb = p_v.tile([P, NT, D], BF16, tag="v")
        p_at = tc.alloc_tile_pool(name="attnw", bufs=3)        # softmax work (-> stage 4)
        p_qk = tc.alloc_tile_pool(name="qkp", bufs=1)          # qAT,hT (-> wave 1)
        qT = p_qk.tile([P, DC, SEQ], BF16, tag="qT")
        hT = p_qk.tile([P, DC, SEQ], BF16, tag="hT")

        p_wq = tc.alloc_tile_pool(name="wqp", bufs=1)          # wa,wb (-> stage 2)
        wa_sb = p_wq.tile([P, DC, DC, P], BF16, tag="wa")
        wb_sb = p_wq.tile([P, DC, D], BF16, tag="wb")

        # ---------- stage 1: load h (host-gathered emb[x]+pos) in both layouts
        # DMA order = stage-2 dependency order: wa col-chunk 0, hT half 0 by
        # ki (first matmul only needs ki=0), rest of wa, hT half 1, wb
        nc.sync.dma_start(out=wa_sb[:, 0, :, :], in_=wa_d[:, 0, :, :])
        for ki in range(DC):
            nc.sync.dma_start(out=hT[:, ki, 0:512], in_=ht_d[:, ki, 0:512])
            if ki < DC - 1:
                nc.sync.dma_start(
                    out=wa_sb[:, ki + 1, :, :], in_=wa_d[:, ki + 1, :, :]
                )
        # wb + early masks before hT half 1: the first four v groups and
        # scores only touch hT[:, :, 0:512]
        nc.sync.dma_start(out=wb_sb[:], in_=wb_d[:].rearrange("(k p) o -> p k o", p=P))

        make_identity(nc, ident_f[:])
        ident_bf = const.tile([P, P], BF16, tag="ident_bf")
        nc.vector.tensor_copy(out=ident_bf[:], in_=ident_f[:])
        ident_f8 = const.tile([P, P], F8, tag="ident_f8")
        nc.vector.tensor_copy(out=ident_f8[:], in_=ident_f[:])

        # ---------- stage 2 + wave 1 merged: qAT, scores/softmax, v ----------
        # all mask loads up front, then the deferred token-major h load
        msk_ts = []
        for i in range(NT):
            msk_t = p_at.tile([P, 512], F8, tag="msk", bufs=NT, name=f"msk{i}")
            msk_ts.append(msk_t)
        for i in range(DC):
            nc.sync.dma_start(out=msk_ts[i][:], in_=msk_d[i])
        for ki in range(DC):
            nc.sync.dma_start(out=hT[:, ki, 512:1024], in_=ht_d[:, ki, 512:1024])
        for i in range(DC, NT):
            nc.sync.dma_start(out=msk_ts[i][:], in_=msk_d[i])
        nc.sync.dma_start(out=h_sb[:], in_=h_d[:])

        attns = [None] * NT
        recips = [None] * NT

        def scores_i(i):
            ws = _window_start(i)
            nw = min(512, (i + 1) * P)  # live window (bf16: any size ok)
            ps_s = psum.tile([P, 512], F32, tag="ps")
            for ki in range(DC):
                nc.tensor.matmul(
                    ps_s[:, :nw],
                    qT[:, ki, i * P : (i + 1) * P],
                    hT[:, ki, ws : ws + nw],
                    start=(ki == 0),
                    stop=False,
                )
            # additive mask folded into the PSUM group: ps += I^T @ mask
            nc.tensor.matmul(
                ps_s[:, :nw], ident_f8[:], msk_ts[i][:, :nw], start=False, stop=True,
            )
            attn = p_at.tile([P, 512], BF16, tag="attn", bufs=NT, name=f"attn{i}")
            denom = small.tile([P, 1], F32, tag="denom")
            # A carries the 1/sqrt(D) factor, so the scores arrive pre-scaled
            nc.scalar.activation(
                out=attn[:, :nw], in_=ps_s[:, :nw], func=AF.Exp,
                bias=0.0, scale=1.0,
                accum_out=denom[:, 0:1],
            )
            recip = small.tile([P, 1], F32, tag="recip", bufs=NT, name=f"recip{i}")
            nc.vector.reciprocal(out=recip[:], in_=denom[:])
            attns[i] = attn
            recips[i] = recip

        # t-major order: all groups needing hT[0:512] first (PE is in-order);
        # scores for the finished t-half run among the v groups so the
        # softmax chain (ACT/DVE) hides behind stage-2 PE work
        for t in range(SEQ // 512):
            for m in range(DC):
                ps = psum.tile([P, 512], F32, tag="ps")
                for ki in range(DC):
                    nc.tensor.matmul(
                        ps[:],
                        wa_sb[:, m, ki, :],
                        hT[:, ki, t * 512 : (t + 1) * 512],
                        start=(ki == 0),
                        stop=(ki == DC - 1),
                    )
                dslc = qT[:, m, t * 512 : (t + 1) * 512]
                if m % 2 == 0:
                    nc.vector.tensor_copy(out=dslc, in_=ps[:])
                else:
                    nc.scalar.copy(out=dslc, in_=ps[:])
            for j in range(4 * t, 4 * t + 4):
                scores_i(j)
                ps = psum.tile([P, 512], F32, tag="ps")
                for ki in range(DC):
                    nc.tensor.matmul(
                        ps[:],
                        hT[:, ki, j * P : (j + 1) * P],
                        wb_sb[:, ki, :],
                        start=(ki == 0),
                        stop=(ki == DC - 1),
                    )
                if j % 2 == 0:
                    nc.scalar.copy(out=v_sb[:, j, :], in_=ps[:])
                else:
                    nc.vector.tensor_copy(out=v_sb[:, j, :], in_=ps[:])

        p_wq.release()
        p_qk.release()

        # ----- right-side pools for FFN phase -----
        whpool = tc.alloc_tile_pool(name="whstream", bufs=wh_bufs, side="right")
        p_h1 = tc.alloc_tile_pool(name="h1p", bufs=1, side="right")
        h1_sb = p_h1.tile([P, NT, D], BF16, tag="h1")
        h1T = p_h1.tile([P, DC, SEQ], BF16, tag="h1T")
        w1_sb = p_h1.tile([P, DC, DFF], BF16, tag="w1")
        nc.sync.dma_start(out=w1_sb[:], in_=w1_d[:].rearrange("(k p) o -> p k o", p=P))

        # ---------- stage 3 wave 2 + stage 4, software-pipelined ----------
        p_st4 = tc.alloc_tile_pool(name="st4", bufs=3)
        attnTs = [None] * NT
        o_ps = [None] * NT

        def w2_a(i):  # attn transposes (bf16) + attnT eviction
            ws = _window_start(i)
            kb0 = ws // P
            nkb = min(DC, i - kb0 + 1)
            pt = psum_t.tile([P, 512], BF16, tag="ptb", bufs=3, name=f"atp{i}")
            for kk in range(nkb):
                nc.tensor.transpose(
                    out=pt[:, kk * P : (kk + 1) * P],
                    in_=attns[i][:, kk * P : (kk + 1) * P],
                    identity=ident_bf[:],
                )
            attnT = p_at.tile([P, 512], BF16, tag="attnT", bufs=3, name=f"attnT{i}")
            nc.scalar.copy(out=attnT[:, : nkb * P], in_=pt[:, : nkb * P])
            attnTs[i] = attnT

        def w2_b(i):  # o matmuls + scale
            ws = _window_start(i)
            kb0 = ws // P
            nkb = min(DC, i - kb0 + 1)
            ps_o = psum.tile([P, 512], F32, tag="ps", name=f"pso{i}")
            for kk in range(nkb):
                nc.tensor.matmul(
                    ps_o[:],
                    attnTs[i][:, kk * P : (kk + 1) * P],
                    v_sb[:, kb0 + kk, :],
                    start=(kk == 0),
                    stop=(kk == nkb - 1),
                )
            o_ps[i] = ps_o

        def s4_ln(j):  # residual + LN1 (o is already fully projected via B)
            r1 = p_st4.tile([P, D], F32, tag="r1", name=f"r1_{j}")
            # fused softmax-normalize + residual: r1 = o_psum * recip + h
            nc.vector.scalar_tensor_tensor(
                out=r1[:], in0=o_ps[j][:], scalar=recips[j][:, 0:1],
                in1=h_sb[:, j, :],
                op0=mybir.AluOpType.mult, op1=mybir.AluOpType.add,
            )
            if bc_bc is not None:
                nc.vector.tensor_add(out=r1[:], in0=r1[:], in1=bc_bc[:])
            stats = small.tile([P, 6], F32, tag="stats")
            nc.vector.bn_stats(out=stats[:], in_=r1[:])
            mv = small.tile([P, 2], F32, tag="mv")
            nc.vector.bn_aggr(out=mv[:], in_=stats[:])
            stdt = small.tile([P, 1], F32, tag="stdt")
            nc.scalar.activation(
                out=stdt[:], in_=mv[:, 1:2], func=AF.Sqrt,
                bias=eps_t[:, 0:1], scale=1.0,
            )
            rstd = small.tile([P, 1], F32, tag="rstd")
            nc.vector.reciprocal(out=rstd[:], in_=stdt[:])
            nc.vector.tensor_scalar(
                out=h1_sb[:, j, :], in0=r1[:],
                scalar1=mv[:, 0:1], scalar2=rstd[:, 0:1],
                op0=mybir.AluOpType.subtract, op1=mybir.AluOpType.mult,
            )
            if g1_bc is not None:
                nc.vector.tensor_mul(out=h1_sb[:, j, :], in0=h1_sb[:, j, :], in1=g1_bc[:])
            if be1_bc is not None:
                nc.vector.tensor_add(out=h1_sb[:, j, :], in0=h1_sb[:, j, :], in1=be1_bc[:])

        def s4_trans(j):  # h1 transposes (bf16) + h1T eviction
            pt3 = psum_t.tile([P, 512], BF16, tag="ptb", bufs=3, name=f"h1p{j}")
            for m in range(DC):
                nc.tensor.transpose(
                    out=pt3[:, m * P : (m + 1) * P],
                    in_=h1_sb[:, j, m * P : (m + 1) * P],
                    identity=ident_bf[:],
                )
            nc.scalar.copy(out=h1T[:, :, j * P : (j + 1) * P], in_=pt3[:])

        def ffn1_group_def_marker(): pass

        def ffn1_group(n, t):
            ps = psum.tile([P, 512], F32, tag="ps", name=f"psf{n}_{t}")
            for ki in range(DC):
                nc.tensor.matmul(
                    ps[:],
                    w1_sb[:, ki, n * P : (n + 1) * P],
                    h1T[:, ki, t * 512 : (t + 1) * 512],
                    start=(ki == 0),
                    stop=(ki == DC - 1),
                )
            fslc = f1T[:, n, t * 512 : (t + 1) * 512]
            if b1_sb is not None:
                nc.scalar.activation(
                    out=fslc, in_=ps[:], func=AF.Relu,
                    bias=b1_sb[:, n : n + 1], scale=1.0,
                )
            elif n % 2 == 0:
                nc.vector.tensor_scalar_max(out=fslc, in0=ps[:], scalar1=0.0)
            else:
                nc.scalar.activation(
                    out=fslc, in_=ps[:], func=AF.Relu, bias=0.0, scale=1.0,
                )

        for k in range(NT + 3):
            if k < NT:
                w2_a(k)
            if 1 <= k < NT + 1:
                w2_b(k - 1)
            if 2 <= k < NT + 2:
                s4_ln(k - 2)
            if 3 <= k:
                s4_trans(k - 3)

        p_st4.release()
        p_at.release()
        p_v.release()
        p_woh.release()

        p_w12 = tc.alloc_tile_pool(name="w12", bufs=1, side="right")
        w2_sb = p_w12.tile([P, FC, D], BF16, tag="w2")
        nc.sync.dma_start(out=w2_sb[:], in_=w2_d[:].rearrange("(k p) o -> p k o", p=P))
        p_f1 = tc.alloc_tile_pool(name="f1p", bufs=1, side="right")
        f1T = p_f1.tile([P, FC, SEQ], BF16, tag="f1T")

        # ---------- stage 6: FFN down + residual + LN2 (pipelined) ----------
        def s6_main(j):
            ps = psum.tile([P, 512], F32, tag="ps", name=f"ps6_{j}")
            for n in range(FC):
                nc.tensor.matmul(
                    ps[:],
                    f1T[:, n, j * P : (j + 1) * P],
                    w2_sb[:, n, :],
                    start=(n == 0),
                    stop=(n == FC - 1),
                )
            r2 = p_f1.tile([P, D], F32, tag="r2", bufs=3, name=f"r2_{j}")
            nc.vector.tensor_add(out=r2[:], in0=h1_sb[:, j, :], in1=ps[:])
            if b2_bc is not None:
                nc.vector.tensor_add(out=r2[:], in0=r2[:], in1=b2_bc[:])
            stats = small.tile([P, 6], F32, tag="stats")
            nc.vector.bn_stats(out=stats[:], in_=r2[:])
            mv = small.tile([P, 2], F32, tag="mv")
            nc.vector.bn_aggr(out=mv[:], in_=stats[:])
            stdt = small.tile([P, 1], F32, tag="stdt")
            # stdt = sqrt((var+eps))/SX so the LN output comes out x SX
            nc.scalar.activation(
                out=stdt[:], in_=mv[:, 1:2], func=AF.Sqrt,
                bias=eps2_t[:, 0:1], scale=1.0 / (SX * SX),
            )
            rstd = small.tile([P, 1], F32, tag="rstd")
            nc.vector.reciprocal(out=rstd[:], in_=stdt[:])
            h2_t = p_f1.tile([P, D], BF16, tag="h2_t", bufs=3, name=f"h2t_{j}")
            nc.vector.tensor_scalar(
                out=h2_t[:], in0=r2[:],
                scalar1=mv[:, 0:1], scalar2=rstd[:, 0:1],
                op0=mybir.AluOpType.subtract, op1=mybir.AluOpType.mult,
            )
            if g2_bc is not None:
                nc.vector.tensor_mul(out=h2_t[:], in0=h2_t[:], in1=g2_bc[:])
            if be2_bc is not None:
                nc.vector.tensor_add(out=h2_t[:], in0=h2_t[:], in1=be2_bc[:])
            return h2_t

        h2ts = [None] * NT

        def s6_trans(j):
            pt = psum_t.tile([P, 512], BF16, tag="ptb", bufs=3, name=f"h2p{j}")
            for m in range(DC):
                nc.tensor.transpose(
                    out=pt[:, m * P : (m + 1) * P],
                    in_=h2ts[j][:, m * P : (m + 1) * P],
                    identity=ident_bf[:],
                )
            # fp8 split: hi = q8(h2T), lo = q8(h2T - hi)
            nc.scalar.copy(out=h2Th[j][:, :, :], in_=pt[:])
            nc.vector.tensor_sub(
                out=h2Tl[j][:, :, :], in0=pt[:], in1=h2Th[j][:, :, :]
            )

        # head chunks for vc=0,1 interleaved into stage-6 so PE fills LN waits
        whh_r = whh_d[:].rearrange("(k p) v -> p k v", p=P)
        whl_r = whl_d[:].rearrange("(k p) v -> p k v", p=P)

        def load_whv(vc, name):
            wh = whpool.tile([P, DC, 512], F8, tag="whv", name=f"{name}h")
            nc.sync.dma_start(out=wh[:], in_=whh_r[:, :, vc * 512 : (vc + 1) * 512])
            wl = whpool.tile([P, DC, 512], F8, tag="whv", name=f"{name}l")
            nc.sync.dma_start(out=wl[:], in_=whl_r[:, :, vc * 512 : (vc + 1) * 512])
            return wh, wl

        NWARM = 5  # head chunks interleaved into stage 6
        whvw = []
        otw = []
        for vc in range(NWARM):
            whvw.append(load_whv(vc, f"whv{vc}"))
            otw.append(opool.tile([P, NT, 512], BF16 if OUT_BF16 else F32,
                                  tag="ot", bufs=NWARM + 1, name=f"otile{vc}"))

        def head_j(whv, otile, j, toggle):
            wh, wl = whv
            ps = psum.tile([P, 512], F32, tag="ps", name=f"psh{toggle}_{j}")
            # 3-term error-corrected fp8, all DoubleRow (contract 256/instr):
            #   xh@wh + xl@wh + xh@wl
            terms = ((h2Th[j], wh), (h2Tl[j], wh), (h2Th[j], wl))
            nterm = len(terms)
            for t_i, (xt, wt) in enumerate(terms):
                for k2 in range(DC // 2):
                    nc.tensor.matmul(
                        ps[:],
                        xt[:, 2 * k2 : 2 * k2 + 2, :],
                        wt[:, 2 * k2 : 2 * k2 + 2, :],
                        start=(t_i == 0 and k2 == 0),
                        stop=(t_i == nterm - 1 and k2 == DC // 2 - 1),
                        perf_mode=DR,
                    )
            if bh_sb_for(toggle) is not None:
                sc = whpool.tile([P, 512], F32, tag="hsc", bufs=2, name=f"hsc{toggle}_{j}")
                nc.scalar.activation(
                    out=sc[:], in_=ps[:], func=AF.Identity, bias=0.0, scale=INV_SXSW,
                )
                nc.vector.tensor_add(out=otile[:, j, :], in0=sc[:], in1=bh_sb_for(toggle)[:])
            elif j % 2 == 0:
                nc.vector.tensor_scalar_mul(out=otile[:, j, :], in0=ps[:], scalar1=INV_SXSW)
            else:
                nc.scalar.activation(
                    out=otile[:, j, :], in_=ps[:], func=AF.Identity,
                    bias=0.0, scale=INV_SXSW,
                )

        _bh_tiles = {}

        def bh_sb_for(key):
            return _bh_tiles.get(key)

        if bh_d is not None:
            for vc in range(NWARM):
                bhv = whpool.tile([P, 512], F32, tag="bh", bufs=2, name=f"bh{vc}")
                nc.sync.dma_start(
                    out=bhv[:], in_=_bcast_ap(bh_d[vc * 512 : (vc + 1) * 512])
                )
                _bh_tiles[vc] = bhv

        for t in range(SEQ // 512):
            for n in range(FC):
                ffn1_group(n, t)
                if t == 1 and n % 2 == 1:
                    h2ts[n // 2] = s6_main(n // 2)

        for k in range(NT + NWARM + 1):
            if 4 <= k < NT:
                h2ts[k] = s6_main(k)
            if 1 <= k <= NT:
                s6_trans(k - 1)
            for w in range(NWARM):
                if 2 + w <= k <= NT + 1 + w:
                    head_j(whvw[w], otw[w], k - 2 - w, w)
        out_rr = out_d[:].rearrange("(j p) v -> p j v", p=P)
        for vc in range(NWARM):
            nc.sync.dma_start(
                out=out_rr[:, :, vc * 512 : (vc + 1) * 512], in_=otw[vc][:]
            )

        p_f1.release()
        p_w12.release()
        p_h1.release()

        # ---------- stage 7: vocab head (vc >= 2) ----------
        out_r = out_d[:].rearrange("(j p) v -> p j v", p=P)
        for vc in range(NWARM, NV):
            whv = load_whv(vc, f"whv{vc}")
            if bh_d is not None:
                bh_bc = whpool.tile([P, 512], F32, tag="bh", bufs=2, name=f"bh{vc}")
                nc.sync.dma_start(
                    out=bh_bc[:], in_=_bcast_ap(bh_d[vc * 512 : (vc + 1) * 512])
                )
                _bh_tiles[vc] = bh_bc
            otile = opool.tile([P, NT, 512], BF16 if OUT_BF16 else F32,
                               tag="ot", bufs=NWARM + 1)
            # split stores so the final drain is short (esp. the last chunk)
            nstore = 4 if vc == NV - 1 else 2
            per = NT // nstore
            for j in range(NT):
                head_j(whv, otile, j, vc)
                if (j + 1) % per == 0:
                    nc.sync.dma_start(
                        out=out_r[:, j + 1 - per : j + 1, vc * 512 : (vc + 1) * 512],
                        in_=otile[:, j + 1 - per : j + 1, :],
                    )

        whpool.release()
        p_h2T.release()
        opool.release()
        psum_t.release()
        psum.release()
        small.release()
        const.release()

    nc.finalize()
    return nc


_PROGRAM_CACHE: dict = {}


def _get_program(flags: dict) -> bass.Bass:
    key = tuple(sorted(flags.items()))
    if key not in _PROGRAM_CACHE:
        _PROGRAM_CACHE[key] = _build_program(flags)
    return _PROGRAM_CACHE[key]


def _prep(x, embed_tab, row_embed, col_embed, Wq, bq, Wk, bk, Wv, bv, Wo, bo,
          ln1_g, ln1_b, W1, b1, W2, b2, ln2_g, ln2_b, Wh, bh):
    """Shared host-side prep: flags, common input map, per-core x shards."""
    f32c = lambda a: np.ascontiguousarray(np.asarray(a, dtype=np.float32))
    x = np.asarray(x)
    B = x.shape[0]
    assert x.shape == (B, SEQ)

    import ml_dtypes
    bfc = lambda a: np.ascontiguousarray(np.asarray(a, dtype=np.float32).astype(ml_dtypes.bfloat16))
    # fused attention weights (f32 host matmuls):
    #   scores = q k^T / sqrt(D) = h (Wq Wk^T / sqrt(D)) h^T   (biases zero)
    #   o = attn @ v @ Wo = attn @ (h (Wv Wo)) + (bv Wo + bo)
    wa = (f32c(Wq) @ f32c(Wk).T) * np.float32(1.0 / math.sqrt(D))
    wb = f32c(Wv) @ f32c(Wo)
    bc = f32c(bv) @ f32c(Wo) + f32c(bo)
    # wa m-major: wa_t[p, m, k, c] = wa[k*P+p, m*P+c]
    wa_t = wa.reshape(DC, P, DC, P).transpose(1, 2, 0, 3)
    arrs = dict(
        wa=bfc(wa_t), wb=bfc(wb),
        w1=bfc(W1), w2=bfc(W2),
    )
    whs = f32c(Wh) * np.float32(SW)
    whh = whs.astype(ml_dtypes.float8_e4m3)
    whl = (whs - whh.astype(np.float32)).astype(ml_dtypes.float8_e4m3)
    arrs["whh"] = np.ascontiguousarray(whh)
    arrs["whl"] = np.ascontiguousarray(whl)
    pos = np.concatenate(
        [np.repeat(f32c(row_embed), GW, axis=0), np.tile(f32c(col_embed), (GH, 1))],
        axis=-1,
    ).astype(np.float32)
    arrs["maskt"] = _mask_tiles().astype(ml_dtypes.float8_e4m3)

    bias_map = dict(
        bc=bc, b1=f32c(b1),
        b2=f32c(b2), bh=f32c(bh),
        be1=f32c(ln1_b),
        # LN2's output is produced pre-scaled by SX; its bias must match
        be2=f32c(ln2_b) * np.float32(SX),
    )
    gain_map = dict(g1=f32c(ln1_g), g2=f32c(ln2_g))
    flags = {k: bool(np.any(v)) for k, v in bias_map.items()}
    flags.update({k: bool(np.any(v != 1.0)) for k, v in gain_map.items()})
    # the QK fusion drops per-row-constant score terms; valid only with
    # zero q/k biases (softmax shift-invariance covers the row-constant part)
    flags["bq"] = bool(np.any(f32c(bq)))
    flags["bk"] = bool(np.any(f32c(bk)))
    for k, v in {**bias_map, **gain_map}.items():
        if flags[k]:
            arrs[k] = v

    # host-side embedding gather + positional add, shipped per core in both
    # layouts: h [P, NT, D] token-major f32, ht [P, DC, SEQ] d-major bf16
    emb = f32c(embed_tab)
    hs, hts = [], []
    for c in range(B):
        h = emb[x[c]] + pos  # [SEQ, D] f32
        hs.append(np.ascontiguousarray(h.reshape(NT, P, D).transpose(1, 0, 2)))
        hts.append(np.ascontiguousarray(
            h.T.reshape(DC, P, SEQ).transpose(1, 0, 2).astype(ml_dtypes.bfloat16)
        ))
    return flags, arrs, hs, hts, B


def kernel(**inputs):
    flags, arrs, hs, hts, B = _prep(**inputs)
    nc = _get_program(flags)
    core_ids = list(range(8))
    in_maps = [{**arrs, "h": hs[c % B], "ht": hts[c % B]} for c in core_ids]
    res = run_bass_kernel_spmd(nc, in_maps, core_ids)
    out = np.stack([res.results[c]["out"] for c in range(B)], axis=0)
    return np.asarray(out, dtype=np.float32)

